# revision 44
# baseline (speedup 1.0000x reference)
"""ButterflyBlock sparse-attention kernel for 8 Trainium2 NeuronCores.

Full inputs in, full output out. The P*B = 32 butterfly blocks are
data-parallel: 4 blocks per core, QKVO weights persistent in SBUF,
chunk gather/scatter done host-side in numpy.

Hardcoded problem shape: x [4, 4096, 1024], D=1024, H=16 heads, dh=64,
CHUNK=256 -> C=16 chunks, pairs a < a^(1<<layer_bit), blocks of L=512.

Schedule: globally software-pipelined emission keeping the PE gap-free.
Attention of block b is interleaved with Q/K/V projections of block b+1
(and deferred Wo groups) as filler work, so the scores->exp->PV chain
never stalls the tensor engine and the PE p-state stays at max clock.
"""

import sys

sys.path.insert(0, "/root/.axon_site/_ro/trn_rl_repo")
sys.path.insert(0, "/opt/trn_rl_repo")

import ml_dtypes
import numpy as np

import concourse.bass as bass
import concourse.bacc as bacc
import concourse.mybir as mybir
import concourse.tile as tile
from concourse.bass_utils import run_bass_kernel_spmd

F32 = mybir.dt.float32
BF16 = mybir.dt.bfloat16

B, N, D = 4, 4096, 1024
H, DH = 16, 64
CHUNK = 256
L = 2 * CHUNK          # 512 tokens per block
NBLK = 4               # blocks per core
NCORES = 8
KC = D // 128          # 8 contraction chunks
LC = L // 128          # 4 token chunks
EXP_FUNC = mybir.ActivationFunctionType.Exp

# v_sb free layout per m-chunk: 16 heads x 64 v cols (pure V).  The
# softmax denominator S is computed by separate 32-col ones-matmuls,
# quad-packed into the four PE column tiles of one PSUM bank, and the
# PV pair (heads 2c, 2c+1) runs as two concurrent 64-col column tiles.
VW = H * 64            # 1024


def _build_nc(has_bq, has_bk, has_bv):
    nc = bacc.Bacc("TRN2", target_bir_lowering=False, debug=False)

    zt = nc.dram_tensor("zt", [NBLK, D, L], BF16, kind="ExternalInput")
    # wq/wk are dc-major: [128, dc, kc*128] so one DMA chunk unlocks a
    # whole projection output group at cold start
    wq = nc.dram_tensor("wq", [128, KC, D], BF16, kind="ExternalInput")
    wk = nc.dram_tensor("wk", [128, KC, D], BF16, kind="ExternalInput")
    # wv/wo are kc-major (moving operands)
    wv = nc.dram_tensor("wv", [128, KC, D], BF16, kind="ExternalInput")
    wo = nc.dram_tensor("wo", [128, KC, D], BF16, kind="ExternalInput")
    ones = nc.dram_tensor("ones", [128, 64], BF16, kind="ExternalInput")
    y = nc.dram_tensor("y", [NBLK, L, D], BF16, kind="ExternalOutput")
    bq = bk = bv = None
    if has_bq:
        bq = nc.dram_tensor("bq", [128, KC], F32, kind="ExternalInput")
    if has_bk:
        bk = nc.dram_tensor("bk", [128, KC], F32, kind="ExternalInput")
    if has_bv:
        bv = nc.dram_tensor("bv", [128, KC], F32, kind="ExternalInput")

    with tile.TileContext(nc) as tc:
        with (
            tc.tile_pool(name="persist", bufs=1) as pp,
            tc.tile_pool(name="ysb", bufs=3) as ypool,
            tc.tile_pool(name="rsb", bufs=2) as rpool,
            tc.tile_pool(name="scps", bufs=2, space="PSUM") as scps,
            tc.tile_pool(name="mmps", bufs=4, space="PSUM") as mmps,
        ):
            # ---- persistent SBUF tiles -------------------------------
            wq_sb = pp.tile([128, KC, D], BF16, tag="wq")
            wk_sb = pp.tile([128, KC, D], BF16, tag="wk")
            wv_sb = pp.tile([128, KC, D], BF16, tag="wv")
            wo_sb = pp.tile([128, KC, D], BF16, tag="wo")
            zt_sb = [pp.tile([128, KC, L], BF16, tag="zt%d" % i, name="zt%d" % i)
                     for i in range(2)]
            q_sb = [pp.tile([128, KC, L], BF16, tag="q%d" % i, name="q%d" % i)
                    for i in range(2)]
            k_sb = [pp.tile([128, KC, L], BF16, tag="k%d" % i, name="k%d" % i)
                    for i in range(2)]
            v_sb = [pp.tile([128, LC, VW], BF16, tag="v%d" % i, name="v%d" % i)
                    for i in range(2)]
            u_sb = [pp.tile([128, KC, L], BF16, tag="u%d" % i, name="u%d" % i)
                    for i in range(2)]
            p_e = [pp.tile([128, LC, 512], BF16, tag="pe%d" % i, name="pe%d" % i)
                   for i in range(4)]
            p_o = [pp.tile([128, LC, 512], BF16, tag="po%d" % i, name="po%d" % i)
                   for i in range(4)]

            # ---- HAM warmup --------------------------------------------
            # the framework preamble + DMA ring startup keeps the PE idle
            # for ~11us; throwaway matmuls on a zeroed tile keep it busy
            # through that window so the HAM clock gate is already at 8/8
            # (2.4 GHz) when the first projection matmul issues.  More are
            # interleaved into the DMA-paced first projection below so the
            # PE never idles long enough to re-throttle.
            wu_sb = pp.tile([128, 512], BF16, tag="wu")
            nc.vector.memset(wu_sb[:], 0.0)
            wu_ps = scps.tile([128, 2, 512], F32, tag="sc")

            def warm(n):
                for _ in range(n):
                    nc.tensor.matmul(
                        wu_ps[:, 0, :], wu_sb[:, 0:128].opt(), wu_sb[:].opt(),
                        start=True, stop=True,
                    )

            warm(10)

            bq_sb = bk_sb = bv_sb = None
            if has_bq:
                bq_sb = pp.tile([128, KC], F32, tag="bq")
                nc.sync.dma_start(bq_sb[:], bq[:])
            if has_bk:
                bk_sb = pp.tile([128, KC], F32, tag="bk")
                nc.sync.dma_start(bk_sb[:], bk[:])
            if has_bv:
                bv_sb = pp.tile([128, KC], F32, tag="bv")
                nc.sync.dma_start(bv_sb[:], bv[:])

            # ---- initial DMAs ----------------------------------------
            # zt block 0 per-kc on the gpsimd queue (fine grain so the
            # first projection matmuls start ~1us in); wq dc-chunks on
            # the sync queue.  Remaining weights + zt follow.
            zt_r = [zt[b].rearrange("(kc p) l -> p kc l", p=128)
                    for b in range(NBLK)]
            # zt block 0 per-kc on the gpsimd queue so the first projection
            # matmuls start as soon as each chunk lands; wq/wk split across
            # the sync + scalar HWDGE rings
            for kc in range(KC):
                nc.gpsimd.dma_start(zt_sb[0][:, kc, :], zt_r[0][:, kc, :])
            for dc in range(KC):
                eng = nc.sync if dc % 2 == 0 else nc.scalar
                eng.dma_start(wq_sb[:, dc, :], wq[:, dc, :])
            for dc in range(KC):
                eng = nc.sync if dc % 2 == 0 else nc.scalar
                eng.dma_start(wk_sb[:, dc, :], wk[:, dc, :])
            for kc in range(KC):
                nc.sync.dma_start(wv_sb[:, kc, :], wv[:, kc, :])
            for kc in range(KC):
                nc.sync.dma_start(wo_sb[:, kc, :], wo[:, kc, :])
            # ones columns for the softmax-sum matmuls
            ones_sb = pp.tile([128, 64], BF16, tag="onesb")
            nc.sync.dma_start(ones_sb[:], ones[:])
            # zt block 1 prefetch (buffer 1, no prior reader)
            nc.gpsimd.dma_start(zt_sb[1][:], zt_r[1])

            # ---- emitters --------------------------------------------
            def qk_group(b, dc, which, warm_fill=0):
                """Q or K projection output-chunk dc of block b."""
                w = wq_sb if which == 0 else wk_sb
                out = q_sb[b % 2] if which == 0 else k_sb[b % 2]
                b_s = bq_sb if which == 0 else bk_sb
                ps = mmps.tile([128, L], F32, tag="mm")
                for kc in range(KC):
                    nc.tensor.matmul(
                        ps[:],
                        w[:, dc, kc * 128:(kc + 1) * 128].opt(),
                        zt_sb[b % 2][:, kc, :].opt(),
                        start=(kc == 0),
                        stop=(kc == KC - 1),
                    )
                    if warm_fill and kc < KC - 1:
                        warm(warm_fill)
                if b_s is not None:
                    nc.scalar.activation(
                        out[:, dc, :], ps[:],
                        mybir.ActivationFunctionType.Identity,
                        bias=b_s[:, dc:dc + 1], scale=1.0,
                    )
                else:
                    nc.vector.tensor_copy(out[:, dc, :], ps[:])

            def v_group(b, g):
                """V projection group g=(lc, nh) of block b."""
                lc, nh = g // 2, g % 2
                ps = mmps.tile([128, 512], F32, tag="mm")
                for kc in range(KC):
                    nc.tensor.matmul(
                        ps[:],
                        zt_sb[b % 2][:, kc, lc * 128:(lc + 1) * 128].opt(),
                        wv_sb[:, kc, nh * 512:(nh + 1) * 512].opt(),
                        start=(kc == 0),
                        stop=(kc == KC - 1),
                    )
                # heads nh*8..nh*8+7, 64 v cols each, contiguous
                nc.vector.tensor_copy(
                    v_sb[b % 2][:, lc, nh * 512:(nh + 1) * 512], ps[:])

            y_rr = [0]

            def wo_group(b, g, split_y=False, y_on_scalar=False):
                """Output projection group g=(lc, eh) of block b.
                dc ascends so the accumulation chases the last u chunks.
                y leaves as bf16, round-robined over two HWDGE rings so the
                final block's writes drain ~2x faster."""
                lc, eh = g // 2, g % 2
                ps = mmps.tile([128, 512], F32, tag="mm")
                for dc in range(KC):
                    nc.tensor.matmul(
                        ps[:],
                        u_sb[b % 2][:, dc, lc * 128:(lc + 1) * 128].opt(),
                        wo_sb[:, dc, eh * 512:(eh + 1) * 512].opt(),
                        start=(dc == 0),
                        stop=(dc == KC - 1),
                    )
                y_sb = ypool.tile([128, 512], BF16, tag="y")
                halves = (0, 256, 512) if split_y else (0, 512)
                for lo, hi in zip(halves, halves[1:]):
                    if y_on_scalar:
                        nc.scalar.copy(y_sb[:, lo:hi], ps[:, lo:hi])
                    else:
                        nc.vector.tensor_copy(y_sb[:, lo:hi], ps[:, lo:hi])
                    eng = (nc.sync, nc.gpsimd)[y_rr[0] % 2]
                    y_rr[0] += 1
                    eng.dma_start(
                        y[b, lc * 128:(lc + 1) * 128,
                          eh * 512 + lo:eh * 512 + hi],
                        y_sb[:, lo:hi],
                    )

            def sc_mg(b, c, mg):
                """Scores chunk-group mg (key chunks 2mg, 2mg+1) for head
                pair c of block b; emits the even-parity exp eagerly and
                the rest after mg1 so the scalar queue drains e-major."""
                t_e = scps.tile([128, 2, 512], F32, tag="sc")
                t_o = scps.tile([128, 2, 512], F32, tag="sc")
                for i in range(2):
                    mc = 2 * mg + i
                    for par, t in ((0, t_e), (1, t_o)):
                        half = par * 64
                        nc.tensor.matmul(
                            t[:, i, :],
                            k_sb[b % 2][half:half + 64, c,
                                        mc * 128:(mc + 1) * 128].opt(),
                            q_sb[b % 2][half:half + 64, c, :].opt(),
                            start=True, stop=True,
                        )
                return t_e, t_o

            def att_phase(b, fillers, nf3=False):
                """nf3: 3 filler slots per head pair instead of 2 -- use when
                the filler list can cover ~24 slots, so the PE never drains
                while the scalar engine (exp, the attention pacer) catches
                up; with fewer fillers the extra slots leave the window
                exp-bound and it runs slower."""
                fi = iter(fillers)

                def F():
                    f = next(fi, None)
                    if f is not None:
                        f()

                ub = u_sb[b % 2]
                rq = [None, None]

                def pv_pair(c):
                    """PV for the head pair (2c, 2c+1) as two concurrent
                    64-col column tiles of one PSUM bank: head 2c -> rows
                    0:64, head 2c+1 -> rows 64:128."""
                    ps = mmps.tile([128, 512], F32, tag="mm")
                    pe_t, po_t = p_e[c % 4], p_o[c % 4]
                    e, o = 2 * c, 2 * c + 1
                    for mc in range(LC):
                        nc.tensor.matmul(
                            ps[0:64, :],
                            v_sb[b % 2][:, mc, e * 64:(e + 1) * 64].opt(),
                            pe_t[:, mc, :].opt(),
                            start=(mc == 0), stop=(mc == LC - 1),
                        )
                        nc.tensor.matmul(
                            ps[64:128, :],
                            v_sb[b % 2][:, mc, o * 64:(o + 1) * 64].opt(),
                            po_t[:, mc, :].opt(),
                            start=(mc == 0), stop=(mc == LC - 1),
                        )
                    return ps

                def s_quad(qd):
                    """Softmax sums for the 4 heads of pairs 2qd, 2qd+1 as
                    four concurrent 32-col column tiles of one PSUM bank;
                    head 4qd+j -> rows 32j:32j+32 (32 copies of S each)."""
                    sp = mmps.tile([128, 512], F32, tag="mm")
                    for mc in range(LC):
                        for j in range(4):
                            c = 2 * qd + j // 2
                            p_t = (p_e if j % 2 == 0 else p_o)[c % 4]
                            nc.tensor.matmul(
                                sp[32 * j:32 * (j + 1), :],
                                ones_sb[:, 0:32].opt(),
                                p_t[:, mc, :].opt(),
                                start=(mc == 0), stop=(mc == LC - 1),
                                tile_position=(0, 32 * j),
                            )
                    return sp

                def recip(qd, sp):
                    r = rpool.tile([128, 512], F32, tag="rq")
                    nc.vector.reciprocal_approx_fast(r[:], sp[:])
                    rq[qd % 2] = r

                def norm_pair(c, ps):
                    """u = PV / S for pair c; the four 32-row strips use the
                    matching S rows of the quad reciprocal."""
                    r = rq[(c // 2) % 2]
                    jo = 64 * (c % 2)
                    nc.vector.tensor_mul(ub[0:32, c, :], ps[0:32, :],
                                         r[jo:jo + 32, :])
                    nc.vector.tensor_mul(ub[32:64, c, :], ps[32:64, :],
                                         r[jo:jo + 32, :])
                    nc.vector.tensor_mul(ub[64:96, c, :], ps[64:96, :],
                                         r[jo + 32:jo + 64, :])
                    nc.vector.tensor_mul(ub[96:128, c, :], ps[96:128, :],
                                         r[jo + 32:jo + 64, :])
                    if has_bv:
                        nc.vector.tensor_scalar_add(
                            ub[:, c, :], ub[:, c, :], bv_sb[:, c:c + 1])

                for c in range(H // 2):
                    pe, po = (p_e[c % 4], p_o[c % 4])
                    t_e0, t_o0 = sc_mg(b, c, 0)
                    nc.scalar.activation(pe[:, 0:2, :], t_e0[:], EXP_FUNC)
                    nc.scalar.activation(po[:, 0:2, :], t_o0[:], EXP_FUNC)
                    if c % 2 == 0 and c >= 2:
                        recip(c // 2 - 1, s_quad(c // 2 - 1))
                    F()
                    t_e1, t_o1 = sc_mg(b, c, 1)
                    nc.scalar.activation(pe[:, 2:4, :], t_e1[:], EXP_FUNC)
                    nc.scalar.activation(po[:, 2:4, :], t_o1[:], EXP_FUNC)
                    if nf3:
                        F()
                    if c >= 2:
                        norm_pair(c - 2, pv_pair(c - 2))
                    F()
                # epilogue: last quad.  pv_pair(6) is runnable immediately
                # (pair 6's p landed an iteration ago); s_quad(3) and
                # pv_pair(7) wait on pair 7's final exps, so they go behind
                # it and behind a filler to keep the in-order queue moving.
                ps6 = pv_pair(6)
                F()
                recip(3, s_quad(3))
                norm_pair(6, ps6)
                F()
                norm_pair(7, pv_pair(7))
                for f in fi:   # drain any leftover fillers
                    f()

            # ---- global emission order -------------------------------
            # cold: block-0 projections (DMA-paced); a few extra warmup
            # matmuls fill the early DMA-arrival gaps so the HAM never
            # sees an idle window
            for dc in range(KC):
                qk_group(0, dc, 0)
            for dc in range(KC):
                qk_group(0, dc, 1)
            for g in range(8):
                v_group(0, g)
            # zt0's last reader (V0) is emitted; buffer 0 may now be
            # refilled with block 2 (emission order IS the dep order)
            nc.gpsimd.dma_start(zt_sb[0][:], zt_r[2])

            # att0 || [Q1, K1, V1] -- 24 fillers over 25 slots (nf3)
            att_phase(0, [lambda dc=dc: qk_group(1, dc, 0) for dc in range(KC)]
                      + [lambda dc=dc: qk_group(1, dc, 1) for dc in range(KC)]
                      + [lambda g=g: v_group(1, g) for g in range(8)],
                      nf3=True)
            # zt1's last reader (V1) emitted; refill buffer 1 with block 3
            nc.gpsimd.dma_start(zt_sb[1][:], zt_r[3])

            # att1 || [Q2, K2, V2, Wo0 g0] -- 25 fillers over 25 slots (nf3)
            att_phase(1, [lambda dc=dc: qk_group(2, dc, 0) for dc in range(KC)]
                      + [lambda dc=dc: qk_group(2, dc, 1) for dc in range(KC)]
                      + [lambda g=g: v_group(2, g) for g in range(8)]
                      + [lambda: wo_group(0, 0)],
                      nf3=True)
            for g in range(1, 8):
                wo_group(0, g, y_on_scalar=True)
            for g in range(5):
                wo_group(1, g, y_on_scalar=True)

            # att2 || [Q3, V3, K3 g0/g1] -- the trailing K3 groups land in
            # the epilogue/drain slots, just ahead of att3's first scores
            att_phase(2, [lambda dc=dc: qk_group(3, dc, 0) for dc in range(KC)]
                      + [lambda g=g: v_group(3, g) for g in range(8)]
                      + [lambda dc=dc: qk_group(3, dc, 1) for dc in range(2)])

            # att3 || [Wo1 spill, K3 rest, Wo2] -- the Wo1 spill groups must
            # all be consumed before att3's first u-normalize write (blocks
            # 1 and 3 share the u buffer); the first norm lands after 5
            # filler slots
            att_phase(3, [lambda g=g: wo_group(1, g) for g in range(5, 8)]
                      + [lambda dc=dc: qk_group(3, dc, 1)
                         for dc in range(2, KC)]
                      + [lambda g=g: wo_group(2, g, y_on_scalar=(g >= 4))
                         for g in range(8)])
            for g in range(8):
                wo_group(3, g, split_y=True, y_on_scalar=True)

    nc.finalize()
    return nc


_NC_CACHE = {}


def _get_nc(flags):
    if flags not in _NC_CACHE:
        _NC_CACHE[flags] = _build_nc(*flags)
    return _NC_CACHE[flags]


def _prep(x, Wq, bq, Wk, bk, Wv, bv, Wo, bo, layer_bit):
    x = np.asarray(x, dtype=np.float32)
    C = N // CHUNK
    ids = np.arange(C)
    partner = ids ^ (1 << int(layer_bit))
    a_idx = ids[ids < partner]
    b_idx = partner[ids < partner]
    P = a_idx.shape[0]

    xr = x.reshape(B, C, CHUNK, D)
    blocks = np.concatenate([xr[:, a_idx], xr[:, b_idx]], axis=2)  # [B,P,L,D]
    blocks = np.ascontiguousarray(
        blocks.transpose(1, 0, 3, 2).reshape(P * B, D, L).astype(ml_dtypes.bfloat16)
    )  # z^T per block
    scale = np.float32(1.0 / np.sqrt(DH))

    def chunkify(vec):  # [D] -> [128, KC] chunk-major per-partition scalars
        return np.ascontiguousarray(
            np.asarray(vec, np.float32).reshape(KC, 128).T
        )

    bf = ml_dtypes.bfloat16

    def dc_major(w):  # [D, D] -> [128, dc, kc*128]
        a = np.asarray(w, np.float32).reshape(KC, 128, KC, 128)
        return np.ascontiguousarray(
            a.transpose(1, 2, 0, 3).reshape(128, KC, D).astype(bf))

    def kc_major(w):  # [D, D] -> [128, kc, D]
        a = np.asarray(w, np.float32).reshape(KC, 128, D)
        return np.ascontiguousarray(a.transpose(1, 0, 2).astype(bf))

    base = {
        "wq": dc_major(np.asarray(Wq, np.float32) * scale),
        "wk": dc_major(Wk),
        "wv": kc_major(Wv),
        "wo": kc_major(Wo),
        "ones": np.ones((128, 64), bf),
    }
    has_bq = bool(np.any(np.asarray(bq))) if bq is not None else False
    has_bk = bool(np.any(np.asarray(bk))) if bk is not None else False
    has_bv = bool(np.any(np.asarray(bv))) if bv is not None else False
    if has_bq:
        base["bq"] = chunkify(np.asarray(bq, np.float32) * scale)
    if has_bk:
        base["bk"] = chunkify(bk)
    if has_bv:
        base["bv"] = chunkify(bv)

    in_maps = []
    for core in range(NCORES):
        m = dict(base)
        m["zt"] = blocks[core * NBLK:(core + 1) * NBLK]
        in_maps.append(m)
    return in_maps, (has_bq, has_bk, has_bv), (a_idx, b_idx, P)


def _gather(results, idxs, bo):
    a_idx, b_idx, P = idxs
    yb = np.concatenate([np.asarray(r["y"], np.float32) for r in results],
                        axis=0)  # [P*B, L, D]
    yb = yb.reshape(P, B, 2, CHUNK, D)
    out = np.empty((B, N // CHUNK, CHUNK, D), np.float32)
    out[:, a_idx] = yb[:, :, 0].transpose(1, 0, 2, 3)
    out[:, b_idx] = yb[:, :, 1].transpose(1, 0, 2, 3)
    out = out.reshape(B, N, D)
    bo = np.asarray(bo, np.float32) if bo is not None else None
    if bo is not None and np.any(bo):
        out = out + bo
    return out


def _run(inputs, trace=False):
    in_maps, flags, idxs = _prep(
        inputs["x"], inputs["Wq"], inputs.get("bq"), inputs["Wk"],
        inputs.get("bk"), inputs["Wv"], inputs.get("bv"), inputs["Wo"],
        inputs.get("bo"), inputs["layer_bit"],
    )
    nc = _get_nc(flags)
    res = run_bass_kernel_spmd(nc, in_maps, list(range(NCORES)), trace=trace)
    out = _gather(res.results, idxs, inputs.get("bo"))
    return out, res


def kernel(**inputs):
    out, _ = _run(inputs, trace=False)
    return out


def kernel_traced(**inputs):
    out, res = _run(inputs, trace=True)
    return out, res



# revision 46
# speedup vs baseline: 1.0109x; 1.0109x over previous
"""ButterflyBlock sparse-attention kernel for 8 Trainium2 NeuronCores.

Full inputs in, full output out. The P*B = 32 butterfly blocks are
data-parallel: 4 blocks per core, QKVO weights persistent in SBUF,
chunk gather/scatter done host-side in numpy.

Hardcoded problem shape: x [4, 4096, 1024], D=1024, H=16 heads, dh=64,
CHUNK=256 -> C=16 chunks, pairs a < a^(1<<layer_bit), blocks of L=512.

Schedule: globally software-pipelined emission keeping the PE gap-free.
Attention of block b is interleaved with Q/K/V projections of block b+1
(and deferred Wo groups) as filler work, so the scores->exp->PV chain
never stalls the tensor engine and the PE p-state stays at max clock.
"""

import sys

sys.path.insert(0, "/root/.axon_site/_ro/trn_rl_repo")
sys.path.insert(0, "/opt/trn_rl_repo")

import ml_dtypes
import numpy as np

import concourse.bass as bass
import concourse.bacc as bacc
import concourse.mybir as mybir
import concourse.tile as tile
from concourse.bass_utils import run_bass_kernel_spmd

F32 = mybir.dt.float32
BF16 = mybir.dt.bfloat16

B, N, D = 4, 4096, 1024
H, DH = 16, 64
CHUNK = 256
L = 2 * CHUNK          # 512 tokens per block
NBLK = 4               # blocks per core
NCORES = 8
KC = D // 128          # 8 contraction chunks
LC = L // 128          # 4 token chunks
EXP_FUNC = mybir.ActivationFunctionType.Exp

# v_sb free layout per m-chunk: 16 heads x 64 v cols (pure V).  The
# softmax denominator S is computed by separate 32-col ones-matmuls,
# quad-packed into the four PE column tiles of one PSUM bank, and the
# PV pair (heads 2c, 2c+1) runs as two concurrent 64-col column tiles.
VW = H * 64            # 1024


def _build_nc(has_bq, has_bk, has_bv):
    nc = bacc.Bacc("TRN2", target_bir_lowering=False, debug=False)

    zt = nc.dram_tensor("zt", [NBLK, D, L], BF16, kind="ExternalInput")
    # wq/wk are dc-major: [128, dc, kc*128] so one DMA chunk unlocks a
    # whole projection output group at cold start
    wq = nc.dram_tensor("wq", [128, KC, D], BF16, kind="ExternalInput")
    wk = nc.dram_tensor("wk", [128, KC, D], BF16, kind="ExternalInput")
    # wv/wo are kc-major (moving operands)
    wv = nc.dram_tensor("wv", [128, KC, D], BF16, kind="ExternalInput")
    wo = nc.dram_tensor("wo", [128, KC, D], BF16, kind="ExternalInput")
    ones = nc.dram_tensor("ones", [128, 64], BF16, kind="ExternalInput")
    y = nc.dram_tensor("y", [NBLK, L, D], BF16, kind="ExternalOutput")
    bq = bk = bv = None
    if has_bq:
        bq = nc.dram_tensor("bq", [128, KC], F32, kind="ExternalInput")
    if has_bk:
        bk = nc.dram_tensor("bk", [128, KC], F32, kind="ExternalInput")
    if has_bv:
        bv = nc.dram_tensor("bv", [128, KC], F32, kind="ExternalInput")

    with tile.TileContext(nc) as tc:
        with (
            tc.tile_pool(name="persist", bufs=1) as pp,
            tc.tile_pool(name="ysb", bufs=3) as ypool,
            tc.tile_pool(name="rsb", bufs=2) as rpool,
            tc.tile_pool(name="scps", bufs=2, space="PSUM") as scps,
            tc.tile_pool(name="mmps", bufs=4, space="PSUM") as mmps,
        ):
            # ---- persistent SBUF tiles -------------------------------
            wq_sb = pp.tile([128, KC, D], BF16, tag="wq")
            wk_sb = pp.tile([128, KC, D], BF16, tag="wk")
            wv_sb = pp.tile([128, KC, D], BF16, tag="wv")
            wo_sb = pp.tile([128, KC, D], BF16, tag="wo")
            zt_sb = [pp.tile([128, KC, L], BF16, tag="zt%d" % i, name="zt%d" % i)
                     for i in range(2)]
            q_sb = [pp.tile([128, KC, L], BF16, tag="q%d" % i, name="q%d" % i)
                    for i in range(2)]
            k_sb = [pp.tile([128, KC, L], BF16, tag="k%d" % i, name="k%d" % i)
                    for i in range(2)]
            v_sb = [pp.tile([128, LC, VW], BF16, tag="v%d" % i, name="v%d" % i)
                    for i in range(2)]
            u_sb = [pp.tile([128, KC, L], BF16, tag="u%d" % i, name="u%d" % i)
                    for i in range(2)]
            p_e = [pp.tile([128, LC, 512], BF16, tag="pe%d" % i, name="pe%d" % i)
                   for i in range(4)]
            p_o = [pp.tile([128, LC, 512], BF16, tag="po%d" % i, name="po%d" % i)
                   for i in range(4)]

            # ---- HAM warmup --------------------------------------------
            # the framework preamble + DMA ring startup keeps the PE idle
            # for ~11us; throwaway matmuls on a zeroed tile keep it busy
            # through that window so the HAM clock gate is already at 8/8
            # (2.4 GHz) when the first projection matmul issues.  More are
            # interleaved into the DMA-paced first projection below so the
            # PE never idles long enough to re-throttle.
            wu_sb = pp.tile([128, 512], BF16, tag="wu")
            nc.vector.memset(wu_sb[:], 0.0)
            wu_ps = scps.tile([128, 2, 512], F32, tag="sc")

            def warm(n):
                for _ in range(n):
                    nc.tensor.matmul(
                        wu_ps[:, 0, :], wu_sb[:, 0:128].opt(), wu_sb[:].opt(),
                        start=True, stop=True,
                    )

            warm(10)

            bq_sb = bk_sb = bv_sb = None
            if has_bq:
                bq_sb = pp.tile([128, KC], F32, tag="bq")
                nc.sync.dma_start(bq_sb[:], bq[:])
            if has_bk:
                bk_sb = pp.tile([128, KC], F32, tag="bk")
                nc.sync.dma_start(bk_sb[:], bk[:])
            if has_bv:
                bv_sb = pp.tile([128, KC], F32, tag="bv")
                nc.sync.dma_start(bv_sb[:], bv[:])

            # ---- initial DMAs ----------------------------------------
            # zt block 0 per-kc on the gpsimd queue (fine grain so the
            # first projection matmuls start ~1us in); wq dc-chunks on
            # the sync queue.  Remaining weights + zt follow.
            zt_r = [zt[b].rearrange("(kc p) l -> p kc l", p=128)
                    for b in range(NBLK)]
            # zt block 0 per-kc on the gpsimd queue so the first projection
            # matmuls start as soon as each chunk lands; wq/wk split across
            # the sync + scalar HWDGE rings
            for kc in range(KC):
                nc.gpsimd.dma_start(zt_sb[0][:, kc, :], zt_r[0][:, kc, :])
            for dc in range(KC):
                eng = nc.sync if dc % 2 == 0 else nc.scalar
                eng.dma_start(wq_sb[:, dc, :], wq[:, dc, :])
            for dc in range(KC):
                eng = nc.sync if dc % 2 == 0 else nc.scalar
                eng.dma_start(wk_sb[:, dc, :], wk[:, dc, :])
            for kc in range(KC):
                nc.sync.dma_start(wv_sb[:, kc, :], wv[:, kc, :])
            for kc in range(KC):
                nc.sync.dma_start(wo_sb[:, kc, :], wo[:, kc, :])
            # ones columns for the softmax-sum matmuls
            ones_sb = pp.tile([128, 64], BF16, tag="onesb")
            nc.sync.dma_start(ones_sb[:], ones[:])
            # zt block 1 prefetch (buffer 1, no prior reader)
            nc.gpsimd.dma_start(zt_sb[1][:], zt_r[1])

            # ---- emitters --------------------------------------------
            def qk_group(b, dc, which, warm_fill=0):
                """Q or K projection output-chunk dc of block b."""
                w = wq_sb if which == 0 else wk_sb
                out = q_sb[b % 2] if which == 0 else k_sb[b % 2]
                b_s = bq_sb if which == 0 else bk_sb
                ps = mmps.tile([128, L], F32, tag="mm")
                for kc in range(KC):
                    nc.tensor.matmul(
                        ps[:],
                        w[:, dc, kc * 128:(kc + 1) * 128].opt(),
                        zt_sb[b % 2][:, kc, :].opt(),
                        start=(kc == 0),
                        stop=(kc == KC - 1),
                    )
                    if warm_fill and kc < KC - 1:
                        warm(warm_fill)
                if b_s is not None:
                    nc.scalar.activation(
                        out[:, dc, :], ps[:],
                        mybir.ActivationFunctionType.Identity,
                        bias=b_s[:, dc:dc + 1], scale=1.0,
                    )
                else:
                    nc.vector.tensor_copy(out[:, dc, :], ps[:])

            def v_group(b, g):
                """V projection group g=(lc, nh) of block b."""
                lc, nh = g // 2, g % 2
                ps = mmps.tile([128, 512], F32, tag="mm")
                for kc in range(KC):
                    nc.tensor.matmul(
                        ps[:],
                        zt_sb[b % 2][:, kc, lc * 128:(lc + 1) * 128].opt(),
                        wv_sb[:, kc, nh * 512:(nh + 1) * 512].opt(),
                        start=(kc == 0),
                        stop=(kc == KC - 1),
                    )
                # heads nh*8..nh*8+7, 64 v cols each, contiguous
                nc.vector.tensor_copy(
                    v_sb[b % 2][:, lc, nh * 512:(nh + 1) * 512], ps[:])

            y_rr = [0]

            def wo_group(b, g, split_y=False, y_on_scalar=False):
                """Output projection group g=(lc, eh) of block b.
                dc ascends so the accumulation chases the last u chunks.
                y leaves as bf16, round-robined over two HWDGE rings so the
                final block's writes drain ~2x faster."""
                lc, eh = g // 2, g % 2
                ps = mmps.tile([128, 512], F32, tag="mm")
                for dc in range(KC):
                    nc.tensor.matmul(
                        ps[:],
                        u_sb[b % 2][:, dc, lc * 128:(lc + 1) * 128].opt(),
                        wo_sb[:, dc, eh * 512:(eh + 1) * 512].opt(),
                        start=(dc == 0),
                        stop=(dc == KC - 1),
                    )
                y_sb = ypool.tile([128, 512], BF16, tag="y")
                halves = (0, 256, 512) if split_y else (0, 512)
                for lo, hi in zip(halves, halves[1:]):
                    if y_on_scalar:
                        nc.scalar.copy(y_sb[:, lo:hi], ps[:, lo:hi])
                    else:
                        nc.vector.tensor_copy(y_sb[:, lo:hi], ps[:, lo:hi])
                    eng = (nc.sync, nc.gpsimd)[y_rr[0] % 2]
                    y_rr[0] += 1
                    eng.dma_start(
                        y[b, lc * 128:(lc + 1) * 128,
                          eh * 512 + lo:eh * 512 + hi],
                        y_sb[:, lo:hi],
                    )

            def sc_mg(b, c, mg):
                """Scores chunk-group mg (key chunks 2mg, 2mg+1) for head
                pair c of block b; emits the even-parity exp eagerly and
                the rest after mg1 so the scalar queue drains e-major."""
                t_e = scps.tile([128, 2, 512], F32, tag="sc")
                t_o = scps.tile([128, 2, 512], F32, tag="sc")
                for i in range(2):
                    mc = 2 * mg + i
                    for par, t in ((0, t_e), (1, t_o)):
                        half = par * 64
                        nc.tensor.matmul(
                            t[:, i, :],
                            k_sb[b % 2][half:half + 64, c,
                                        mc * 128:(mc + 1) * 128].opt(),
                            q_sb[b % 2][half:half + 64, c, :].opt(),
                            start=True, stop=True,
                        )
                return t_e, t_o

            def att_phase(b, fillers, nf3=False):
                """nf3: 3 filler slots per head pair instead of 2 -- use when
                the filler list can cover ~24 slots, so the PE never drains
                while the scalar engine (exp, the attention pacer) catches
                up; with fewer fillers the extra slots leave the window
                exp-bound and it runs slower."""
                fi = iter(fillers)

                def F():
                    f = next(fi, None)
                    if f is not None:
                        f()

                ub = u_sb[b % 2]
                rq = [None, None]

                def pv_pair(c):
                    """PV for the head pair (2c, 2c+1) as two concurrent
                    64-col column tiles of one PSUM bank: head 2c -> rows
                    0:64, head 2c+1 -> rows 64:128."""
                    ps = mmps.tile([128, 512], F32, tag="mm")
                    pe_t, po_t = p_e[c % 4], p_o[c % 4]
                    e, o = 2 * c, 2 * c + 1
                    for mc in range(LC):
                        nc.tensor.matmul(
                            ps[0:64, :],
                            v_sb[b % 2][:, mc, e * 64:(e + 1) * 64].opt(),
                            pe_t[:, mc, :].opt(),
                            start=(mc == 0), stop=(mc == LC - 1),
                        )
                        nc.tensor.matmul(
                            ps[64:128, :],
                            v_sb[b % 2][:, mc, o * 64:(o + 1) * 64].opt(),
                            po_t[:, mc, :].opt(),
                            start=(mc == 0), stop=(mc == LC - 1),
                        )
                    return ps

                def s_quad(qd):
                    """Softmax sums for the 4 heads of pairs 2qd, 2qd+1 as
                    four concurrent 32-col column tiles of one PSUM bank;
                    head 4qd+j -> rows 32j:32j+32 (32 copies of S each)."""
                    sp = mmps.tile([128, 512], F32, tag="mm")
                    for mc in range(LC):
                        for j in range(4):
                            c = 2 * qd + j // 2
                            p_t = (p_e if j % 2 == 0 else p_o)[c % 4]
                            nc.tensor.matmul(
                                sp[32 * j:32 * (j + 1), :],
                                ones_sb[:, 0:32].opt(),
                                p_t[:, mc, :].opt(),
                                start=(mc == 0), stop=(mc == LC - 1),
                                tile_position=(0, 32 * j),
                            )
                    return sp

                def recip(qd, sp):
                    r = rpool.tile([128, 512], F32, tag="rq")
                    nc.vector.reciprocal_approx_fast(r[:], sp[:])
                    rq[qd % 2] = r

                def norm_pair(c, ps):
                    """u = PV / S for pair c; the four 32-row strips use the
                    matching S rows of the quad reciprocal."""
                    r = rq[(c // 2) % 2]
                    jo = 64 * (c % 2)
                    nc.vector.tensor_mul(ub[0:32, c, :], ps[0:32, :],
                                         r[jo:jo + 32, :])
                    nc.vector.tensor_mul(ub[32:64, c, :], ps[32:64, :],
                                         r[jo:jo + 32, :])
                    nc.vector.tensor_mul(ub[64:96, c, :], ps[64:96, :],
                                         r[jo + 32:jo + 64, :])
                    nc.vector.tensor_mul(ub[96:128, c, :], ps[96:128, :],
                                         r[jo + 32:jo + 64, :])
                    if has_bv:
                        nc.vector.tensor_scalar_add(
                            ub[:, c, :], ub[:, c, :], bv_sb[:, c:c + 1])

                for c in range(H // 2):
                    pe, po = (p_e[c % 4], p_o[c % 4])
                    t_e0, t_o0 = sc_mg(b, c, 0)
                    nc.scalar.activation(pe[:, 0:2, :], t_e0[:], EXP_FUNC)
                    nc.scalar.activation(po[:, 0:2, :], t_o0[:], EXP_FUNC)
                    if c % 2 == 0 and c >= 2:
                        recip(c // 2 - 1, s_quad(c // 2 - 1))
                    F()
                    t_e1, t_o1 = sc_mg(b, c, 1)
                    nc.scalar.activation(pe[:, 2:4, :], t_e1[:], EXP_FUNC)
                    nc.scalar.activation(po[:, 2:4, :], t_o1[:], EXP_FUNC)
                    if nf3:
                        F()
                    if c >= 2:
                        norm_pair(c - 2, pv_pair(c - 2))
                    F()
                # epilogue: last quad
                recip(3, s_quad(3))
                norm_pair(6, pv_pair(6))
                F()
                norm_pair(7, pv_pair(7))
                for f in fi:   # drain any leftover fillers
                    f()

            # ---- global emission order -------------------------------
            # cold: block-0 projections (DMA-paced); a few extra warmup
            # matmuls fill the early DMA-arrival gaps so the HAM never
            # sees an idle window
            for dc in range(KC):
                qk_group(0, dc, 0)
            for dc in range(KC):
                qk_group(0, dc, 1)
            for g in range(8):
                v_group(0, g)
            # zt0's last reader (V0) is emitted; buffer 0 may now be
            # refilled with block 2 (emission order IS the dep order)
            nc.gpsimd.dma_start(zt_sb[0][:], zt_r[2])

            # att0 || [Q1, K1]
            att_phase(0, [lambda dc=dc: qk_group(1, dc, 0) for dc in range(KC)]
                      + [lambda dc=dc: qk_group(1, dc, 1) for dc in range(KC)])
            for g in range(8):
                v_group(1, g)
            # zt1's last reader (V1) emitted; refill buffer 1 with block 3
            nc.gpsimd.dma_start(zt_sb[1][:], zt_r[3])
            for g in range(8):
                wo_group(0, g, y_on_scalar=True)

            # att1 || [Q2, K2]
            att_phase(1, [lambda dc=dc: qk_group(2, dc, 0) for dc in range(KC)]
                      + [lambda dc=dc: qk_group(2, dc, 1) for dc in range(KC)])
            for g in range(8):
                v_group(2, g)
            for g in range(5):
                wo_group(1, g, y_on_scalar=True)

            # att2 || [Q3, V3, K3 g0/g1] -- the trailing K3 groups land in
            # the epilogue/drain slots, just ahead of att3's first scores
            att_phase(2, [lambda dc=dc: qk_group(3, dc, 0) for dc in range(KC)]
                      + [lambda g=g: v_group(3, g) for g in range(8)]
                      + [lambda dc=dc: qk_group(3, dc, 1) for dc in range(2)])

            # att3 || [Wo1 spill, K3 rest, Wo2] -- the Wo1 spill groups must
            # all be consumed before att3's first u-normalize write (blocks
            # 1 and 3 share the u buffer); the first norm lands after 5
            # filler slots
            att_phase(3, [lambda g=g: wo_group(1, g) for g in range(5, 8)]
                      + [lambda dc=dc: qk_group(3, dc, 1)
                         for dc in range(2, KC)]
                      + [lambda g=g: wo_group(2, g, y_on_scalar=(g >= 4))
                         for g in range(8)])
            for g in range(8):
                wo_group(3, g, split_y=True, y_on_scalar=True)

    nc.finalize()
    return nc


_NC_CACHE = {}


def _get_nc(flags):
    if flags not in _NC_CACHE:
        _NC_CACHE[flags] = _build_nc(*flags)
    return _NC_CACHE[flags]


def _prep(x, Wq, bq, Wk, bk, Wv, bv, Wo, bo, layer_bit):
    x = np.asarray(x, dtype=np.float32)
    C = N // CHUNK
    ids = np.arange(C)
    partner = ids ^ (1 << int(layer_bit))
    a_idx = ids[ids < partner]
    b_idx = partner[ids < partner]
    P = a_idx.shape[0]

    xr = x.reshape(B, C, CHUNK, D)
    blocks = np.concatenate([xr[:, a_idx], xr[:, b_idx]], axis=2)  # [B,P,L,D]
    blocks = np.ascontiguousarray(
        blocks.transpose(1, 0, 3, 2).reshape(P * B, D, L).astype(ml_dtypes.bfloat16)
    )  # z^T per block
    scale = np.float32(1.0 / np.sqrt(DH))

    def chunkify(vec):  # [D] -> [128, KC] chunk-major per-partition scalars
        return np.ascontiguousarray(
            np.asarray(vec, np.float32).reshape(KC, 128).T
        )

    bf = ml_dtypes.bfloat16

    def dc_major(w):  # [D, D] -> [128, dc, kc*128]
        a = np.asarray(w, np.float32).reshape(KC, 128, KC, 128)
        return np.ascontiguousarray(
            a.transpose(1, 2, 0, 3).reshape(128, KC, D).astype(bf))

    def kc_major(w):  # [D, D] -> [128, kc, D]
        a = np.asarray(w, np.float32).reshape(KC, 128, D)
        return np.ascontiguousarray(a.transpose(1, 0, 2).astype(bf))

    base = {
        "wq": dc_major(np.asarray(Wq, np.float32) * scale),
        "wk": dc_major(Wk),
        "wv": kc_major(Wv),
        "wo": kc_major(Wo),
        "ones": np.ones((128, 64), bf),
    }
    has_bq = bool(np.any(np.asarray(bq))) if bq is not None else False
    has_bk = bool(np.any(np.asarray(bk))) if bk is not None else False
    has_bv = bool(np.any(np.asarray(bv))) if bv is not None else False
    if has_bq:
        base["bq"] = chunkify(np.asarray(bq, np.float32) * scale)
    if has_bk:
        base["bk"] = chunkify(bk)
    if has_bv:
        base["bv"] = chunkify(bv)

    in_maps = []
    for core in range(NCORES):
        m = dict(base)
        m["zt"] = blocks[core * NBLK:(core + 1) * NBLK]
        in_maps.append(m)
    return in_maps, (has_bq, has_bk, has_bv), (a_idx, b_idx, P)


def _gather(results, idxs, bo):
    a_idx, b_idx, P = idxs
    yb = np.concatenate([np.asarray(r["y"], np.float32) for r in results],
                        axis=0)  # [P*B, L, D]
    yb = yb.reshape(P, B, 2, CHUNK, D)
    out = np.empty((B, N // CHUNK, CHUNK, D), np.float32)
    out[:, a_idx] = yb[:, :, 0].transpose(1, 0, 2, 3)
    out[:, b_idx] = yb[:, :, 1].transpose(1, 0, 2, 3)
    out = out.reshape(B, N, D)
    bo = np.asarray(bo, np.float32) if bo is not None else None
    if bo is not None and np.any(bo):
        out = out + bo
    return out


def _run(inputs, trace=False):
    in_maps, flags, idxs = _prep(
        inputs["x"], inputs["Wq"], inputs.get("bq"), inputs["Wk"],
        inputs.get("bk"), inputs["Wv"], inputs.get("bv"), inputs["Wo"],
        inputs.get("bo"), inputs["layer_bit"],
    )
    nc = _get_nc(flags)
    res = run_bass_kernel_spmd(nc, in_maps, list(range(NCORES)), trace=trace)
    out = _gather(res.results, idxs, inputs.get("bo"))
    return out, res


def kernel(**inputs):
    out, _ = _run(inputs, trace=False)
    return out


def kernel_traced(**inputs):
    out, res = _run(inputs, trace=True)
    return out, res



# revision 47
# speedup vs baseline: 1.0147x; 1.0037x over previous
"""ButterflyBlock sparse-attention kernel for 8 Trainium2 NeuronCores.

Full inputs in, full output out. The P*B = 32 butterfly blocks are
data-parallel: 4 blocks per core, QKVO weights persistent in SBUF,
chunk gather/scatter done host-side in numpy.

Hardcoded problem shape: x [4, 4096, 1024], D=1024, H=16 heads, dh=64,
CHUNK=256 -> C=16 chunks, pairs a < a^(1<<layer_bit), blocks of L=512.

Schedule: globally software-pipelined emission keeping the PE gap-free.
Attention of block b is interleaved with Q/K/V projections of block b+1
(and deferred Wo groups) as filler work, so the scores->exp->PV chain
never stalls the tensor engine and the PE p-state stays at max clock.
"""

import sys

sys.path.insert(0, "/root/.axon_site/_ro/trn_rl_repo")
sys.path.insert(0, "/opt/trn_rl_repo")

import ml_dtypes
import numpy as np

import concourse.bass as bass
import concourse.bacc as bacc
import concourse.mybir as mybir
import concourse.tile as tile
from concourse.bass_utils import run_bass_kernel_spmd

F32 = mybir.dt.float32
BF16 = mybir.dt.bfloat16

B, N, D = 4, 4096, 1024
H, DH = 16, 64
CHUNK = 256
L = 2 * CHUNK          # 512 tokens per block
NBLK = 4               # blocks per core
NCORES = 8
KC = D // 128          # 8 contraction chunks
LC = L // 128          # 4 token chunks
EXP_FUNC = mybir.ActivationFunctionType.Exp

# v_sb free layout per m-chunk: 16 heads x 64 v cols (pure V).  The
# softmax denominator S is computed by separate 32-col ones-matmuls,
# quad-packed into the four PE column tiles of one PSUM bank, and the
# PV pair (heads 2c, 2c+1) runs as two concurrent 64-col column tiles.
VW = H * 64            # 1024


def _build_nc(has_bq, has_bk, has_bv):
    nc = bacc.Bacc("TRN2", target_bir_lowering=False, debug=False)

    zt = nc.dram_tensor("zt", [NBLK, D, L], BF16, kind="ExternalInput")
    # wq/wk are dc-major: [128, dc, kc*128] so one DMA chunk unlocks a
    # whole projection output group at cold start
    wq = nc.dram_tensor("wq", [128, KC, D], BF16, kind="ExternalInput")
    wk = nc.dram_tensor("wk", [128, KC, D], BF16, kind="ExternalInput")
    # wv/wo are kc-major (moving operands)
    wv = nc.dram_tensor("wv", [128, KC, D], BF16, kind="ExternalInput")
    wo = nc.dram_tensor("wo", [128, KC, D], BF16, kind="ExternalInput")
    ones = nc.dram_tensor("ones", [128, 64], BF16, kind="ExternalInput")
    y = nc.dram_tensor("y", [NBLK, L, D], BF16, kind="ExternalOutput")
    bq = bk = bv = None
    if has_bq:
        bq = nc.dram_tensor("bq", [128, KC], F32, kind="ExternalInput")
    if has_bk:
        bk = nc.dram_tensor("bk", [128, KC], F32, kind="ExternalInput")
    if has_bv:
        bv = nc.dram_tensor("bv", [128, KC], F32, kind="ExternalInput")

    with tile.TileContext(nc) as tc:
        with (
            tc.tile_pool(name="persist", bufs=1) as pp,
            tc.tile_pool(name="ysb", bufs=3) as ypool,
            tc.tile_pool(name="rsb", bufs=2) as rpool,
            tc.tile_pool(name="scps", bufs=2, space="PSUM") as scps,
            tc.tile_pool(name="mmps", bufs=4, space="PSUM") as mmps,
        ):
            # ---- persistent SBUF tiles -------------------------------
            wq_sb = pp.tile([128, KC, D], BF16, tag="wq")
            wk_sb = pp.tile([128, KC, D], BF16, tag="wk")
            wv_sb = pp.tile([128, KC, D], BF16, tag="wv")
            wo_sb = pp.tile([128, KC, D], BF16, tag="wo")
            zt_sb = [pp.tile([128, KC, L], BF16, tag="zt%d" % i, name="zt%d" % i)
                     for i in range(2)]
            q_sb = [pp.tile([128, KC, L], BF16, tag="q%d" % i, name="q%d" % i)
                    for i in range(2)]
            k_sb = [pp.tile([128, KC, L], BF16, tag="k%d" % i, name="k%d" % i)
                    for i in range(2)]
            v_sb = [pp.tile([128, LC, VW], BF16, tag="v%d" % i, name="v%d" % i)
                    for i in range(2)]
            u_sb = [pp.tile([128, KC, L], BF16, tag="u%d" % i, name="u%d" % i)
                    for i in range(2)]
            p_e = [pp.tile([128, LC, 512], BF16, tag="pe%d" % i, name="pe%d" % i)
                   for i in range(4)]
            p_o = [pp.tile([128, LC, 512], BF16, tag="po%d" % i, name="po%d" % i)
                   for i in range(4)]

            # ---- HAM warmup --------------------------------------------
            # the framework preamble + DMA ring startup keeps the PE idle
            # for ~11us; throwaway matmuls on a zeroed tile keep it busy
            # through that window so the HAM clock gate is already at 8/8
            # (2.4 GHz) when the first projection matmul issues.  More are
            # interleaved into the DMA-paced first projection below so the
            # PE never idles long enough to re-throttle.
            wu_sb = pp.tile([128, 512], BF16, tag="wu")
            nc.vector.memset(wu_sb[:], 0.0)
            wu_ps = scps.tile([128, 2, 512], F32, tag="sc")

            def warm(n):
                for _ in range(n):
                    nc.tensor.matmul(
                        wu_ps[:, 0, :], wu_sb[:, 0:128].opt(), wu_sb[:].opt(),
                        start=True, stop=True,
                    )

            warm(10)

            bq_sb = bk_sb = bv_sb = None
            if has_bq:
                bq_sb = pp.tile([128, KC], F32, tag="bq")
                nc.sync.dma_start(bq_sb[:], bq[:])
            if has_bk:
                bk_sb = pp.tile([128, KC], F32, tag="bk")
                nc.sync.dma_start(bk_sb[:], bk[:])
            if has_bv:
                bv_sb = pp.tile([128, KC], F32, tag="bv")
                nc.sync.dma_start(bv_sb[:], bv[:])

            # ---- initial DMAs ----------------------------------------
            # zt block 0 per-kc on the gpsimd queue (fine grain so the
            # first projection matmuls start ~1us in); wq dc-chunks on
            # the sync queue.  Remaining weights + zt follow.
            zt_r = [zt[b].rearrange("(kc p) l -> p kc l", p=128)
                    for b in range(NBLK)]
            # zt block 0 per-kc on the gpsimd queue so the first projection
            # matmuls start as soon as each chunk lands; wq/wk split across
            # the sync + scalar HWDGE rings
            for kc in range(KC):
                nc.gpsimd.dma_start(zt_sb[0][:, kc, :], zt_r[0][:, kc, :])
            for dc in range(KC):
                eng = nc.sync if dc % 2 == 0 else nc.scalar
                eng.dma_start(wq_sb[:, dc, :], wq[:, dc, :])
            for dc in range(KC):
                eng = nc.sync if dc % 2 == 0 else nc.scalar
                eng.dma_start(wk_sb[:, dc, :], wk[:, dc, :])
            for kc in range(KC):
                nc.sync.dma_start(wv_sb[:, kc, :], wv[:, kc, :])
            for kc in range(KC):
                nc.sync.dma_start(wo_sb[:, kc, :], wo[:, kc, :])
            # ones columns for the softmax-sum matmuls
            ones_sb = pp.tile([128, 64], BF16, tag="onesb")
            nc.sync.dma_start(ones_sb[:], ones[:])
            # zt block 1 prefetch (buffer 1, no prior reader)
            nc.gpsimd.dma_start(zt_sb[1][:], zt_r[1])

            # ---- emitters --------------------------------------------
            def qk_group(b, dc, which, warm_fill=0):
                """Q or K projection output-chunk dc of block b."""
                w = wq_sb if which == 0 else wk_sb
                out = q_sb[b % 2] if which == 0 else k_sb[b % 2]
                b_s = bq_sb if which == 0 else bk_sb
                ps = mmps.tile([128, L], F32, tag="mm")
                for kc in range(KC):
                    nc.tensor.matmul(
                        ps[:],
                        w[:, dc, kc * 128:(kc + 1) * 128].opt(),
                        zt_sb[b % 2][:, kc, :].opt(),
                        start=(kc == 0),
                        stop=(kc == KC - 1),
                    )
                    if warm_fill and kc < KC - 1:
                        warm(warm_fill)
                if b_s is not None:
                    nc.scalar.activation(
                        out[:, dc, :], ps[:],
                        mybir.ActivationFunctionType.Identity,
                        bias=b_s[:, dc:dc + 1], scale=1.0,
                    )
                else:
                    nc.vector.tensor_copy(out[:, dc, :], ps[:])

            def v_group(b, g):
                """V projection group g=(lc, nh) of block b."""
                lc, nh = g // 2, g % 2
                ps = mmps.tile([128, 512], F32, tag="mm")
                for kc in range(KC):
                    nc.tensor.matmul(
                        ps[:],
                        zt_sb[b % 2][:, kc, lc * 128:(lc + 1) * 128].opt(),
                        wv_sb[:, kc, nh * 512:(nh + 1) * 512].opt(),
                        start=(kc == 0),
                        stop=(kc == KC - 1),
                    )
                # heads nh*8..nh*8+7, 64 v cols each, contiguous
                nc.vector.tensor_copy(
                    v_sb[b % 2][:, lc, nh * 512:(nh + 1) * 512], ps[:])

            y_rr = [0]

            def wo_group(b, g, split_y=False, y_on_scalar=False):
                """Output projection group g=(lc, eh) of block b.
                dc ascends so the accumulation chases the last u chunks.
                y leaves as bf16, round-robined over two HWDGE rings so the
                final block's writes drain ~2x faster."""
                lc, eh = g // 2, g % 2
                ps = mmps.tile([128, 512], F32, tag="mm")
                for dc in range(KC):
                    nc.tensor.matmul(
                        ps[:],
                        u_sb[b % 2][:, dc, lc * 128:(lc + 1) * 128].opt(),
                        wo_sb[:, dc, eh * 512:(eh + 1) * 512].opt(),
                        start=(dc == 0),
                        stop=(dc == KC - 1),
                    )
                y_sb = ypool.tile([128, 512], BF16, tag="y")
                halves = (0, 256, 512) if split_y else (0, 512)
                for lo, hi in zip(halves, halves[1:]):
                    if y_on_scalar:
                        nc.scalar.copy(y_sb[:, lo:hi], ps[:, lo:hi])
                    else:
                        nc.vector.tensor_copy(y_sb[:, lo:hi], ps[:, lo:hi])
                    eng = (nc.sync, nc.gpsimd)[y_rr[0] % 2]
                    y_rr[0] += 1
                    eng.dma_start(
                        y[b, lc * 128:(lc + 1) * 128,
                          eh * 512 + lo:eh * 512 + hi],
                        y_sb[:, lo:hi],
                    )

            def sc_mg(b, c, mg):
                """Scores chunk-group mg (key chunks 2mg, 2mg+1) for head
                pair c of block b; emits the even-parity exp eagerly and
                the rest after mg1 so the scalar queue drains e-major."""
                t_e = scps.tile([128, 2, 512], F32, tag="sc")
                t_o = scps.tile([128, 2, 512], F32, tag="sc")
                for i in range(2):
                    mc = 2 * mg + i
                    for par, t in ((0, t_e), (1, t_o)):
                        half = par * 64
                        nc.tensor.matmul(
                            t[:, i, :],
                            k_sb[b % 2][half:half + 64, c,
                                        mc * 128:(mc + 1) * 128].opt(),
                            q_sb[b % 2][half:half + 64, c, :].opt(),
                            start=True, stop=True,
                        )
                return t_e, t_o

            def att_phase(b, fillers, nf3=False):
                """nf3: 3 filler slots per head pair instead of 2 -- use when
                the filler list can cover ~24 slots, so the PE never drains
                while the scalar engine (exp, the attention pacer) catches
                up; with fewer fillers the extra slots leave the window
                exp-bound and it runs slower."""
                fi = iter(fillers)

                def F():
                    f = next(fi, None)
                    if f is not None:
                        f()

                ub = u_sb[b % 2]
                rq = [None, None]

                def pv_pair(c):
                    """PV for the head pair (2c, 2c+1) as two concurrent
                    64-col column tiles of one PSUM bank: head 2c -> rows
                    0:64, head 2c+1 -> rows 64:128."""
                    ps = mmps.tile([128, 512], F32, tag="mm")
                    pe_t, po_t = p_e[c % 4], p_o[c % 4]
                    e, o = 2 * c, 2 * c + 1
                    for mc in range(LC):
                        nc.tensor.matmul(
                            ps[0:64, :],
                            v_sb[b % 2][:, mc, e * 64:(e + 1) * 64].opt(),
                            pe_t[:, mc, :].opt(),
                            start=(mc == 0), stop=(mc == LC - 1),
                        )
                        nc.tensor.matmul(
                            ps[64:128, :],
                            v_sb[b % 2][:, mc, o * 64:(o + 1) * 64].opt(),
                            po_t[:, mc, :].opt(),
                            start=(mc == 0), stop=(mc == LC - 1),
                        )
                    return ps

                def s_quad(qd):
                    """Softmax sums for the 4 heads of pairs 2qd, 2qd+1 as
                    four concurrent 32-col column tiles of one PSUM bank;
                    head 4qd+j -> rows 32j:32j+32 (32 copies of S each)."""
                    sp = mmps.tile([128, 512], F32, tag="mm")
                    for mc in range(LC):
                        for j in range(4):
                            c = 2 * qd + j // 2
                            p_t = (p_e if j % 2 == 0 else p_o)[c % 4]
                            nc.tensor.matmul(
                                sp[32 * j:32 * (j + 1), :],
                                ones_sb[:, 0:32].opt(),
                                p_t[:, mc, :].opt(),
                                start=(mc == 0), stop=(mc == LC - 1),
                                tile_position=(0, 32 * j),
                            )
                    return sp

                def recip(qd, sp):
                    r = rpool.tile([128, 512], F32, tag="rq")
                    nc.vector.reciprocal_approx_fast(r[:], sp[:])
                    rq[qd % 2] = r

                def norm_pair(c, ps):
                    """u = PV / S for pair c; the four 32-row strips use the
                    matching S rows of the quad reciprocal."""
                    r = rq[(c // 2) % 2]
                    jo = 64 * (c % 2)
                    nc.vector.tensor_mul(ub[0:32, c, :], ps[0:32, :],
                                         r[jo:jo + 32, :])
                    nc.vector.tensor_mul(ub[32:64, c, :], ps[32:64, :],
                                         r[jo:jo + 32, :])
                    nc.vector.tensor_mul(ub[64:96, c, :], ps[64:96, :],
                                         r[jo + 32:jo + 64, :])
                    nc.vector.tensor_mul(ub[96:128, c, :], ps[96:128, :],
                                         r[jo + 32:jo + 64, :])
                    if has_bv:
                        nc.vector.tensor_scalar_add(
                            ub[:, c, :], ub[:, c, :], bv_sb[:, c:c + 1])

                for c in range(H // 2):
                    pe, po = (p_e[c % 4], p_o[c % 4])
                    t_e0, t_o0 = sc_mg(b, c, 0)
                    nc.scalar.activation(pe[:, 0:2, :], t_e0[:], EXP_FUNC)
                    nc.scalar.activation(po[:, 0:2, :], t_o0[:], EXP_FUNC)
                    if c % 2 == 0 and c >= 2:
                        recip(c // 2 - 1, s_quad(c // 2 - 1))
                    F()
                    t_e1, t_o1 = sc_mg(b, c, 1)
                    nc.scalar.activation(pe[:, 2:4, :], t_e1[:], EXP_FUNC)
                    nc.scalar.activation(po[:, 2:4, :], t_o1[:], EXP_FUNC)
                    if nf3:
                        F()
                    if c >= 2:
                        norm_pair(c - 2, pv_pair(c - 2))
                    F()
                # epilogue: last quad
                recip(3, s_quad(3))
                norm_pair(6, pv_pair(6))
                F()
                norm_pair(7, pv_pair(7))
                for f in fi:   # drain any leftover fillers
                    f()

            # ---- global emission order -------------------------------
            # cold: block-0 projections (DMA-paced); a few extra warmup
            # matmuls fill the early DMA-arrival gaps so the HAM never
            # sees an idle window
            # dc0 is paced by the zt chunk arrivals (~1.2us apart); two
            # warmup matmuls after each chunk's matmul keep the PE busy
            # enough that the HAM clock gate fires early and stays at 8/8,
            # so the dense dc1+ stretch runs at 2.4 GHz instead of 1.2
            for dc in range(KC):
                qk_group(0, dc, 0, warm_fill=(2 if dc == 0 else 0))
            for dc in range(KC):
                qk_group(0, dc, 1)
            for g in range(8):
                v_group(0, g)
            # zt0's last reader (V0) is emitted; buffer 0 may now be
            # refilled with block 2 (emission order IS the dep order)
            nc.gpsimd.dma_start(zt_sb[0][:], zt_r[2])

            # att0 || [Q1, K1]
            att_phase(0, [lambda dc=dc: qk_group(1, dc, 0) for dc in range(KC)]
                      + [lambda dc=dc: qk_group(1, dc, 1) for dc in range(KC)])
            for g in range(8):
                v_group(1, g)
            # zt1's last reader (V1) emitted; refill buffer 1 with block 3
            nc.gpsimd.dma_start(zt_sb[1][:], zt_r[3])
            for g in range(8):
                wo_group(0, g, y_on_scalar=True)

            # att1 || [Q2, K2]
            att_phase(1, [lambda dc=dc: qk_group(2, dc, 0) for dc in range(KC)]
                      + [lambda dc=dc: qk_group(2, dc, 1) for dc in range(KC)])
            for g in range(8):
                v_group(2, g)
            for g in range(5):
                wo_group(1, g, y_on_scalar=True)

            # att2 || [Q3, V3, K3 g0/g1] -- the trailing K3 groups land in
            # the epilogue/drain slots, just ahead of att3's first scores
            att_phase(2, [lambda dc=dc: qk_group(3, dc, 0) for dc in range(KC)]
                      + [lambda g=g: v_group(3, g) for g in range(8)]
                      + [lambda dc=dc: qk_group(3, dc, 1) for dc in range(2)])

            # att3 || [Wo1 spill, K3 rest, Wo2] -- the Wo1 spill groups must
            # all be consumed before att3's first u-normalize write (blocks
            # 1 and 3 share the u buffer); the first norm lands after 5
            # filler slots
            att_phase(3, [lambda g=g: wo_group(1, g) for g in range(5, 8)]
                      + [lambda dc=dc: qk_group(3, dc, 1)
                         for dc in range(2, KC)]
                      + [lambda g=g: wo_group(2, g, y_on_scalar=(g >= 4))
                         for g in range(8)])
            for g in range(8):
                wo_group(3, g, split_y=True, y_on_scalar=True)

    nc.finalize()
    return nc


_NC_CACHE = {}


def _get_nc(flags):
    if flags not in _NC_CACHE:
        _NC_CACHE[flags] = _build_nc(*flags)
    return _NC_CACHE[flags]


def _prep(x, Wq, bq, Wk, bk, Wv, bv, Wo, bo, layer_bit):
    x = np.asarray(x, dtype=np.float32)
    C = N // CHUNK
    ids = np.arange(C)
    partner = ids ^ (1 << int(layer_bit))
    a_idx = ids[ids < partner]
    b_idx = partner[ids < partner]
    P = a_idx.shape[0]

    xr = x.reshape(B, C, CHUNK, D)
    blocks = np.concatenate([xr[:, a_idx], xr[:, b_idx]], axis=2)  # [B,P,L,D]
    blocks = np.ascontiguousarray(
        blocks.transpose(1, 0, 3, 2).reshape(P * B, D, L).astype(ml_dtypes.bfloat16)
    )  # z^T per block
    scale = np.float32(1.0 / np.sqrt(DH))

    def chunkify(vec):  # [D] -> [128, KC] chunk-major per-partition scalars
        return np.ascontiguousarray(
            np.asarray(vec, np.float32).reshape(KC, 128).T
        )

    bf = ml_dtypes.bfloat16

    def dc_major(w):  # [D, D] -> [128, dc, kc*128]
        a = np.asarray(w, np.float32).reshape(KC, 128, KC, 128)
        return np.ascontiguousarray(
            a.transpose(1, 2, 0, 3).reshape(128, KC, D).astype(bf))

    def kc_major(w):  # [D, D] -> [128, kc, D]
        a = np.asarray(w, np.float32).reshape(KC, 128, D)
        return np.ascontiguousarray(a.transpose(1, 0, 2).astype(bf))

    base = {
        "wq": dc_major(np.asarray(Wq, np.float32) * scale),
        "wk": dc_major(Wk),
        "wv": kc_major(Wv),
        "wo": kc_major(Wo),
        "ones": np.ones((128, 64), bf),
    }
    has_bq = bool(np.any(np.asarray(bq))) if bq is not None else False
    has_bk = bool(np.any(np.asarray(bk))) if bk is not None else False
    has_bv = bool(np.any(np.asarray(bv))) if bv is not None else False
    if has_bq:
        base["bq"] = chunkify(np.asarray(bq, np.float32) * scale)
    if has_bk:
        base["bk"] = chunkify(bk)
    if has_bv:
        base["bv"] = chunkify(bv)

    in_maps = []
    for core in range(NCORES):
        m = dict(base)
        m["zt"] = blocks[core * NBLK:(core + 1) * NBLK]
        in_maps.append(m)
    return in_maps, (has_bq, has_bk, has_bv), (a_idx, b_idx, P)


def _gather(results, idxs, bo):
    a_idx, b_idx, P = idxs
    yb = np.concatenate([np.asarray(r["y"], np.float32) for r in results],
                        axis=0)  # [P*B, L, D]
    yb = yb.reshape(P, B, 2, CHUNK, D)
    out = np.empty((B, N // CHUNK, CHUNK, D), np.float32)
    out[:, a_idx] = yb[:, :, 0].transpose(1, 0, 2, 3)
    out[:, b_idx] = yb[:, :, 1].transpose(1, 0, 2, 3)
    out = out.reshape(B, N, D)
    bo = np.asarray(bo, np.float32) if bo is not None else None
    if bo is not None and np.any(bo):
        out = out + bo
    return out


def _run(inputs, trace=False):
    in_maps, flags, idxs = _prep(
        inputs["x"], inputs["Wq"], inputs.get("bq"), inputs["Wk"],
        inputs.get("bk"), inputs["Wv"], inputs.get("bv"), inputs["Wo"],
        inputs.get("bo"), inputs["layer_bit"],
    )
    nc = _get_nc(flags)
    res = run_bass_kernel_spmd(nc, in_maps, list(range(NCORES)), trace=trace)
    out = _gather(res.results, idxs, inputs.get("bo"))
    return out, res


def kernel(**inputs):
    out, _ = _run(inputs, trace=False)
    return out


def kernel_traced(**inputs):
    out, res = _run(inputs, trace=True)
    return out, res

